# revision 4
# baseline (speedup 1.0000x reference)
"""TRN2 Bass kernel for fused MHA (softmax-over-query quirk) + out-proj + residual + LayerNorm.

Problem shapes (hardcoded): tokens [4,2048,1024], Wq/Wk [16,1024,64], Wv [16,1024,64],
Wo [1024,1024], gamma/beta [1024]. Output [4,2048,1024] fp32.

Sharding: 8 cores, core c owns (batch b=c//2, S-half jc=c%2) of the OUTPUT rows.
No collectives. Each core computes, for its batch b:
  qT[dk,i] (full S), kT[dk,j] (its half), V[i,dv] (full S) in bf16,
  scores^T[i,j] = q_i.k_j (PSUM fp32), e = exp(scores/8) (bf16),
  heads^T[dv,j] + rowsum row via a ones-column appended to V,
  multi^T = heads^T / rowsum, out = multi @ Wo + tokens, LayerNorm rows.

Math done in bf16 matmuls with fp32 PSUM accumulation; residual + LN in fp32.
Validated against fp32 reference: max abs err ~2e-4 (output absmax ~5).
"""

import numpy as np
import ml_dtypes

BF16 = ml_dtypes.bfloat16

B, S, D, H, DK, DV = 4, 2048, 1024, 16, 64, 64
NCORES = 8
NPAIR = 8     # head pairs
NKC = 8       # D // 128 contraction chunks
NIC = 16      # S // 128 i-chunks
JW = 1024     # j columns per core (S/2)
NJCH = 8      # JW // 128
LN_EPS = 1e-5

_CACHE = {}


def _build_nc():
    import concourse.tile as tile
    from concourse import bacc, mybir

    F32 = mybir.dt.float32
    BF = mybir.dt.bfloat16
    Exp = mybir.ActivationFunctionType.Exp
    Square = mybir.ActivationFunctionType.Square
    Sqrt = mybir.ActivationFunctionType.Sqrt
    mult = mybir.AluOpType.mult
    add = mybir.AluOpType.add
    AX = mybir.AxisListType.X

    nc = bacc.Bacc(
        "TRN2",
        target_bir_lowering=False,
        debug=False,
        enable_asserts=False,
        num_devices=NCORES,
    )

    # DRAM I/O (per-core views; host prepares layouts)
    tokT_d = nc.dram_tensor("tokT", (128, NKC, S), BF, kind="ExternalInput").ap()
    tokTj_d = nc.dram_tensor("tokTj", (128, NKC, JW), BF, kind="ExternalInput").ap()
    wq_d = nc.dram_tensor("wq", (128, NKC, H * DK), BF, kind="ExternalInput").ap()
    wk_d = nc.dram_tensor("wk", (128, NKC, H * DK), BF, kind="ExternalInput").ap()
    wv_d = nc.dram_tensor("wv", (128, NKC, H * DV), BF, kind="ExternalInput").ap()
    wo_d = nc.dram_tensor("wo", (128, NKC, D), BF, kind="ExternalInput").ap()
    tokres_d = nc.dram_tensor("tokres", (128, NJCH, D), F32, kind="ExternalInput").ap()
    gamma_d = nc.dram_tensor("gamma_bc", (128, D), F32, kind="ExternalInput").ap()
    beta_d = nc.dram_tensor("beta_bc", (128, D), F32, kind="ExternalInput").ap()
    out_d = nc.dram_tensor("out", (128, NJCH, D), F32, kind="ExternalOutput").ap()

    from contextlib import ExitStack

    with tile.TileContext(nc) as tc, ExitStack() as stack:
        persist = stack.enter_context(tc.tile_pool(name="persist", bufs=1))
        qT_sb = persist.tile([128, NPAIR, S], BF)          # [pair-dk, pr, i]
        kT_sb = persist.tile([128, NPAIR, JW], BF)         # [pair-dk, pr, j]
        v_sb = persist.tile([128, NIC, H, DV + 1], BF)     # [i%128, ic, h, dv|ones]
        multiT_sb = persist.tile([128, NKC, JW], BF)       # [hv%128, hv//128, j]
        gamma_sb = persist.tile([128, D], F32)
        beta_sb = persist.tile([128, D], F32)
        ones_bf = persist.tile([1, DV], BF)
        eps_sb = persist.tile([128, 1], F32)

        nc.sync.dma_start(gamma_sb[:], gamma_d[:])
        nc.sync.dma_start(beta_sb[:], beta_d[:])
        nc.vector.memset(ones_bf[:], 1.0)
        nc.vector.memset(eps_sb[:], LN_EPS)
        # ones column of the extended V (one strided memset per i-chunk)
        for ic in range(NIC):
            nc.vector.memset(v_sb[:, ic, :, DV : DV + 1], 1.0)

        # ---------------- Phase A: projections ----------------
        with (
            tc.tile_pool(name="pa", bufs=1) as pa,
            tc.tile_pool(name="psA", bufs=4, space="PSUM") as psA,
        ):
            tokT_sb = pa.tile([128, NKC, S], BF)
            tokTj_sb = pa.tile([128, NKC, JW], BF)
            wq_sb = pa.tile([128, NKC, H * DK], BF)
            wk_sb = pa.tile([128, NKC, H * DK], BF)
            wv_sb = pa.tile([128, NKC, H * DV], BF)
            for kc in range(NKC):
                nc.sync.dma_start(tokT_sb[:, kc], tokT_d[:, kc])
                nc.sync.dma_start(tokTj_sb[:, kc], tokTj_d[:, kc])
                nc.sync.dma_start(wq_sb[:, kc], wq_d[:, kc])
                nc.sync.dma_start(wk_sb[:, kc], wk_d[:, kc])
                nc.sync.dma_start(wv_sb[:, kc], wv_d[:, kc])

            # qT: [pair-dk(128), i(2048)] per pair, from lhsT=Wq pair cols, rhs=tokens^T
            for pr in range(NPAIR):
                psq = [
                    psA.tile([128, 1024], F32, tag="pa_ps", name=f"psq{pr}_{t}")
                    for t in range(2)
                ]
                for kc in range(NKC):
                    lhsT = wq_sb[:, kc, pr * 128 : (pr + 1) * 128]
                    for nb in range(4):
                        nc.tensor.matmul(
                            psq[nb // 2][:, (nb % 2) * 512 : (nb % 2 + 1) * 512],
                            lhsT,
                            tokT_sb[:, kc, nb * 512 : (nb + 1) * 512],
                            start=(kc == 0),
                            stop=(kc == NKC - 1),
                        )
                for t in range(2):
                    nc.vector.tensor_copy(
                        out=qT_sb[:, pr, t * 1024 : (t + 1) * 1024], in_=psq[t]
                    )

            # kT: [pair-dk(128), j(1024)] per pair (this core's j-half)
            for pr in range(NPAIR):
                psk = psA.tile([128, 1024], F32, tag="pa_ps", name=f"psk{pr}")
                for kc in range(NKC):
                    lhsT = wk_sb[:, kc, pr * 128 : (pr + 1) * 128]
                    for jb in range(2):
                        nc.tensor.matmul(
                            psk[:, jb * 512 : (jb + 1) * 512],
                            lhsT,
                            tokTj_sb[:, kc, jb * 512 : (jb + 1) * 512],
                            start=(kc == 0),
                            stop=(kc == NKC - 1),
                        )
                nc.vector.tensor_copy(out=kT_sb[:, pr], in_=psk[:])

            # V: [i(2048 in 16 chunks), h*dv] natural
            for ic in range(NIC):
                psv = psA.tile([128, 1024], F32, tag="pa_ps", name=f"psv{ic}")
                for kc in range(NKC):
                    lhsT = tokT_sb[:, kc, ic * 128 : (ic + 1) * 128]
                    for nb in range(2):
                        nc.tensor.matmul(
                            psv[:, nb * 512 : (nb + 1) * 512],
                            lhsT,
                            wv_sb[:, kc, nb * 512 : (nb + 1) * 512],
                            start=(kc == 0),
                            stop=(kc == NKC - 1),
                        )
                nc.vector.tensor_copy(
                    out=v_sb[:, ic, :, 0:DV],
                    in_=psv.rearrange("p (h v) -> p h v", h=H),
                )

        # Phase C inputs (DMA overlaps attention; SBUF reuses phase A space)
        pc = stack.enter_context(tc.tile_pool(name="pc", bufs=1))
        wo_sb = pc.tile([128, NKC, D], BF)
        tokres_sb = pc.tile([128, NJCH, D], F32)
        nc.sync.dma_start(wo_sb[:], wo_d[:])
        nc.sync.dma_start(tokres_sb[:], tokres_d[:])

        # ---------------- Phase B: attention per head pair ----------------
        with (
            tc.tile_pool(name="pe", bufs=6) as pe_pool,
            tc.tile_pool(name="pn", bufs=2) as pn_pool,
            tc.tile_pool(name="psS", bufs=2, space="PSUM") as psS,
            tc.tile_pool(name="psAcc", bufs=2, space="PSUM") as psAcc,
        ):
            for pr in range(NPAIR):
                h0, h1 = 2 * pr, 2 * pr + 1
                acc = [
                    psAcc.tile([DV + 1, JW], F32, tag="acc", name=f"acc{pr}_{hh}")
                    for hh in range(2)
                ]
                for ic in range(NIC):
                    ps_s = [
                        psS.tile([128, JW], F32, tag="sc", name=f"ps_s{pr}_{ic}_{hh}")
                        for hh in range(2)
                    ]
                    # scores^T for both heads; row-tiled (K=64 at partitions 0/64)
                    for hh in range(2):
                        lhsT = qT_sb[
                            hh * 64 : (hh + 1) * 64, pr, ic * 128 : (ic + 1) * 128
                        ]
                        for jb in range(2):
                            nc.tensor.matmul(
                                ps_s[hh][:, jb * 512 : (jb + 1) * 512],
                                lhsT,
                                kT_sb[hh * 64 : (hh + 1) * 64, pr, jb * 512 : (jb + 1) * 512],
                                start=True,
                                stop=True,
                            )
                    for hh in range(2):
                        eT = pe_pool.tile([128, JW], BF, tag="eT", name=f"eT{pr}_{ic}_{hh}")
                        nc.scalar.activation(eT[:], ps_s[hh][:], Exp, scale=0.125)
                        for jb in range(2):
                            nc.tensor.matmul(
                                acc[hh][:, jb * 512 : (jb + 1) * 512],
                                v_sb[:, ic, 2 * pr + hh, :],
                                eT[:, jb * 512 : (jb + 1) * 512],
                                start=(ic == 0),
                                stop=(ic == NIC - 1),
                            )
                # normalize: multi^T[h] = heads^T / rowsum
                for hh in range(2):
                    h = 2 * pr + hh
                    hraw = pn_pool.tile([DV + 1, JW], F32, tag="hraw", name=f"hraw{h}")
                    nc.vector.tensor_copy(out=hraw[:], in_=acc[hh][:])
                    recip = pn_pool.tile([1, JW], F32, tag="recip", name=f"recip{h}")
                    nc.vector.reciprocal(recip[:], hraw[DV : DV + 1, :])
                    recip_bf = pn_pool.tile([1, JW], BF, tag="rbf", name=f"rbf{h}")
                    nc.vector.tensor_copy(out=recip_bf[:], in_=recip[:])
                    ps_bc = psS.tile([DV, JW], F32, tag="sc", name=f"ps_bc{h}")
                    for jb in range(2):
                        nc.tensor.matmul(
                            ps_bc[:, jb * 512 : (jb + 1) * 512],
                            ones_bf[:],
                            recip_bf[:, jb * 512 : (jb + 1) * 512],
                            start=True,
                            stop=True,
                        )
                    if hh == 0:
                        nc.vector.tensor_tensor(
                            multiT_sb[0:64, h // 2, :], hraw[0:DV, :], ps_bc[:], mult
                        )
                    else:
                        tmp64 = pn_pool.tile([DV, JW], BF, tag="tmp64", name=f"tmp{h}")
                        nc.vector.tensor_tensor(
                            tmp64[:], hraw[0:DV, :], ps_bc[:], mult
                        )
                        nc.sync.dma_start(
                            out=multiT_sb[64:128, h // 2, :], in_=tmp64[:]
                        )

        # ---------------- Phase C: out-proj + residual + LayerNorm ----------------
        with (
            tc.tile_pool(name="pC", bufs=2) as pC,
            tc.tile_pool(name="pStats", bufs=8) as pStats,
            tc.tile_pool(name="psC", bufs=2, space="PSUM") as psC,
        ):
            for jch in range(NJCH):
                ps_o = psC.tile([128, D], F32, tag="po", name=f"ps_o{jch}")
                for kc in range(NKC):
                    lhsT = multiT_sb[:, kc, jch * 128 : (jch + 1) * 128]
                    for nb in range(2):
                        nc.tensor.matmul(
                            ps_o[:, nb * 512 : (nb + 1) * 512],
                            lhsT,
                            wo_sb[:, kc, nb * 512 : (nb + 1) * 512],
                            start=(kc == 0),
                            stop=(kc == NKC - 1),
                        )
                x_sb = pC.tile([128, D], F32, tag="x", name=f"x{jch}")
                nc.vector.tensor_tensor(x_sb[:], ps_o[:], tokres_sb[:, jch, :], add)
                sum_t = pStats.tile([128, 1], F32, tag="sum", name=f"sum{jch}")
                nc.vector.reduce_sum(sum_t[:], x_sb[:], axis=AX)
                negmean = pStats.tile([128, 1], F32, tag="nm", name=f"nm{jch}")
                nc.vector.tensor_scalar_mul(negmean[:], sum_t[:], -1.0 / D)
                nc.vector.tensor_scalar_add(x_sb[:], x_sb[:], negmean[:])
                sq_sb = pC.tile([128, D], F32, tag="sq", name=f"sq{jch}")
                ssq = pStats.tile([128, 1], F32, tag="ssq", name=f"ssq{jch}")
                nc.scalar.activation(sq_sb[:], x_sb[:], Square, accum_out=ssq[:])
                std_t = pStats.tile([128, 1], F32, tag="std", name=f"std{jch}")
                nc.scalar.activation(std_t[:], ssq[:], Sqrt, bias=eps_sb[:], scale=1.0 / D)
                rstd = pStats.tile([128, 1], F32, tag="rstd", name=f"rstd{jch}")
                nc.vector.reciprocal(rstd[:], std_t[:])
                out_sb = pC.tile([128, D], F32, tag="out", name=f"out{jch}")
                nc.vector.tensor_scalar_mul(out_sb[:], x_sb[:], rstd[:])
                nc.vector.tensor_tensor(out_sb[:], out_sb[:], gamma_sb[:], mult)
                nc.vector.tensor_tensor(out_sb[:], out_sb[:], beta_sb[:], add)
                nc.sync.dma_start(out_d[:, jch], out_sb[:])

    nc.compile()
    return nc


def _prep_inputs(tokens, Wq, Wk, Wv, Wo, gamma, beta):
    """Host-side layout prep. Returns per-core input maps."""
    tokens = np.ascontiguousarray(np.asarray(tokens, dtype=np.float32))
    # weights -> [p, kc, n] with row index kc*128+p
    def rows128(a):  # [1024, N] -> [128, 8, N]
        return np.ascontiguousarray(
            a.reshape(NKC, 128, a.shape[-1]).transpose(1, 0, 2)
        )

    wq_all = rows128(np.asarray(Wq).transpose(1, 0, 2).reshape(D, H * DK).astype(BF16))
    wk_all = rows128(np.asarray(Wk).transpose(1, 0, 2).reshape(D, H * DK).astype(BF16))
    wv_all = rows128(np.asarray(Wv).transpose(1, 0, 2).reshape(D, H * DV).astype(BF16))
    wo_all = rows128(np.asarray(Wo).astype(BF16))
    gamma_bc = np.ascontiguousarray(
        np.broadcast_to(np.asarray(gamma, np.float32), (128, D))
    )
    beta_bc = np.ascontiguousarray(
        np.broadcast_to(np.asarray(beta, np.float32), (128, D))
    )

    tokT_by_b = []
    for b in range(B):
        tokT_by_b.append(rows128(tokens[b].T.astype(BF16)))  # [128, 8, 2048]

    in_maps = []
    for c in range(NCORES):
        b, jc = c // 2, c % 2
        tokT = tokT_by_b[b]
        tokTj = np.ascontiguousarray(tokT[:, :, jc * JW : (jc + 1) * JW])
        tokres = np.ascontiguousarray(
            tokens[b, jc * JW : (jc + 1) * JW]
            .reshape(NJCH, 128, D)
            .transpose(1, 0, 2)
        )
        in_maps.append(
            {
                "tokT": tokT,
                "tokTj": tokTj,
                "wq": wq_all,
                "wk": wk_all,
                "wv": wv_all,
                "wo": wo_all,
                "tokres": tokres,
                "gamma_bc": gamma_bc,
                "beta_bc": beta_bc,
            }
        )
    return in_maps


def run(inputs, trace=False, tmpdir=None):
    """Run on hardware; returns (output, BassKernelResults)."""
    from concourse.bass_utils import run_bass_kernel_spmd

    if "nc" not in _CACHE:
        _CACHE["nc"] = _build_nc()
    nc = _CACHE["nc"]
    in_maps = _prep_inputs(**inputs)
    res = run_bass_kernel_spmd(
        nc, in_maps, core_ids=list(range(NCORES)), trace=trace, tmpdir=tmpdir
    )
    out = np.empty((B, S, D), np.float32)
    for c in range(NCORES):
        b, jc = c // 2, c % 2
        o = res.results[c]["out"]  # [128, 8, 1024]
        out[b, jc * JW : (jc + 1) * JW] = (
            o.transpose(1, 0, 2).reshape(JW, D)
        )
    return out, res


def kernel(tokens, Wq, Wk, Wv, Wo, gamma, beta):
    out, _ = run(
        dict(tokens=tokens, Wq=Wq, Wk=Wk, Wv=Wv, Wo=Wo, gamma=gamma, beta=beta)
    )
    return out


# revision 5
# speedup vs baseline: 1.0024x; 1.0024x over previous
"""TRN2 Bass kernel for fused MHA (softmax-over-query quirk) + out-proj + residual + LayerNorm.

Problem shapes (hardcoded): tokens [4,2048,1024], Wq/Wk [16,1024,64], Wv [16,1024,64],
Wo [1024,1024], gamma/beta [1024]. Output [4,2048,1024] fp32.

Sharding: 8 cores, core c owns (batch b=c//2, S-half jc=c%2) of the OUTPUT rows.
No collectives. Each core computes, for its batch b:
  qT[dk,i] (full S), kT[dk,j] (its half), V[i,dv] (full S) in bf16,
  scores^T[i,j] = q_i.k_j (PSUM fp32), e = exp(scores/8) (bf16),
  heads^T[dv,j] + rowsum row via a ones-column appended to V,
  multi^T = heads^T / rowsum, out = multi @ Wo + tokens, LayerNorm rows.

Math done in bf16 matmuls with fp32 PSUM accumulation; residual + LN in fp32.
Validated against fp32 reference: max abs err ~2e-4 (output absmax ~5).
"""

import numpy as np
import ml_dtypes

BF16 = ml_dtypes.bfloat16

B, S, D, H, DK, DV = 4, 2048, 1024, 16, 64, 64
NCORES = 8
NPAIR = 8     # head pairs
NKC = 8       # D // 128 contraction chunks
NIC = 16      # S // 128 i-chunks
JW = 1024     # j columns per core (S/2)
NJCH = 8      # JW // 128
LN_EPS = 1e-5

_CACHE = {}


def _build_nc():
    import concourse.tile as tile
    from concourse import bacc, mybir

    F32 = mybir.dt.float32
    BF = mybir.dt.bfloat16
    Exp = mybir.ActivationFunctionType.Exp
    Square = mybir.ActivationFunctionType.Square
    Sqrt = mybir.ActivationFunctionType.Sqrt
    mult = mybir.AluOpType.mult
    add = mybir.AluOpType.add
    AX = mybir.AxisListType.X

    nc = bacc.Bacc(
        "TRN2",
        target_bir_lowering=False,
        debug=False,
        enable_asserts=False,
        num_devices=NCORES,
    )

    # DRAM I/O (per-core views; host prepares layouts)
    tokT_d = nc.dram_tensor("tokT", (128, NKC, S), BF, kind="ExternalInput").ap()
    tokTj_d = nc.dram_tensor("tokTj", (128, NKC, JW), BF, kind="ExternalInput").ap()
    wq_d = nc.dram_tensor("wq", (128, NKC, H * DK), BF, kind="ExternalInput").ap()
    wk_d = nc.dram_tensor("wk", (128, NKC, H * DK), BF, kind="ExternalInput").ap()
    wv_d = nc.dram_tensor("wv", (128, NKC, H * DV), BF, kind="ExternalInput").ap()
    wo_d = nc.dram_tensor("wo", (128, NKC, D), BF, kind="ExternalInput").ap()
    tokres_d = nc.dram_tensor("tokres", (128, NJCH, D), F32, kind="ExternalInput").ap()
    gamma_d = nc.dram_tensor("gamma_bc", (128, D), F32, kind="ExternalInput").ap()
    beta_d = nc.dram_tensor("beta_bc", (128, D), F32, kind="ExternalInput").ap()
    out_d = nc.dram_tensor("out", (128, NJCH, D), F32, kind="ExternalOutput").ap()

    from contextlib import ExitStack

    with tile.TileContext(nc) as tc, ExitStack() as stack:
        persist = stack.enter_context(tc.tile_pool(name="persist", bufs=1))
        qT_sb = persist.tile([128, NPAIR, S], BF)          # [pair-dk, pr, i]
        kT_sb = persist.tile([128, NPAIR, JW], BF)         # [pair-dk, pr, j]
        v_sb = persist.tile([128, NIC, H, DV + 1], BF)     # [i%128, ic, h, dv|ones]
        multiT_sb = persist.tile([128, NKC, JW], BF)       # [hv%128, hv//128, j]
        gamma_sb = persist.tile([128, D], F32)
        beta_sb = persist.tile([128, D], F32)
        ones_bf = persist.tile([1, DV], BF)
        eps_sb = persist.tile([128, 1], F32)

        nc.sync.dma_start(gamma_sb[:], gamma_d[:])
        nc.sync.dma_start(beta_sb[:], beta_d[:])
        nc.vector.memset(ones_bf[:], 1.0)
        nc.vector.memset(eps_sb[:], LN_EPS)
        # ones column of the extended V (one strided memset per i-chunk)
        for ic in range(NIC):
            nc.vector.memset(v_sb[:, ic, :, DV : DV + 1], 1.0)

        # ---------------- Phase A: projections ----------------
        with (
            tc.tile_pool(name="pa", bufs=1) as pa,
            tc.tile_pool(name="psA", bufs=4, space="PSUM") as psA,
        ):
            tokT_sb = pa.tile([128, NKC, S], BF)
            tokTj_sb = pa.tile([128, NKC, JW], BF)
            wq_sb = pa.tile([128, NKC, H * DK], BF)
            wk_sb = pa.tile([128, NKC, H * DK], BF)
            wv_sb = pa.tile([128, NKC, H * DV], BF)
            for kc in range(NKC):
                nc.sync.dma_start(tokT_sb[:, kc], tokT_d[:, kc])
                nc.sync.dma_start(tokTj_sb[:, kc], tokTj_d[:, kc])
                nc.sync.dma_start(wq_sb[:, kc], wq_d[:, kc])
                nc.sync.dma_start(wk_sb[:, kc], wk_d[:, kc])
                nc.sync.dma_start(wv_sb[:, kc], wv_d[:, kc])

            # qT: [pair-dk(128), i(2048)] per pair, from lhsT=Wq pair cols, rhs=tokens^T
            for pr in range(NPAIR):
                psq = [
                    psA.tile([128, 1024], F32, tag="pa_ps", name=f"psq{pr}_{t}")
                    for t in range(2)
                ]
                for kc in range(NKC):
                    lhsT = wq_sb[:, kc, pr * 128 : (pr + 1) * 128]
                    for nb in range(4):
                        nc.tensor.matmul(
                            psq[nb // 2][:, (nb % 2) * 512 : (nb % 2 + 1) * 512],
                            lhsT,
                            tokT_sb[:, kc, nb * 512 : (nb + 1) * 512],
                            start=(kc == 0),
                            stop=(kc == NKC - 1),
                        )
                for t in range(2):
                    nc.vector.tensor_copy(
                        out=qT_sb[:, pr, t * 1024 : (t + 1) * 1024], in_=psq[t]
                    )

            # kT: [pair-dk(128), j(1024)] per pair (this core's j-half)
            for pr in range(NPAIR):
                psk = psA.tile([128, 1024], F32, tag="pa_ps", name=f"psk{pr}")
                for kc in range(NKC):
                    lhsT = wk_sb[:, kc, pr * 128 : (pr + 1) * 128]
                    for jb in range(2):
                        nc.tensor.matmul(
                            psk[:, jb * 512 : (jb + 1) * 512],
                            lhsT,
                            tokTj_sb[:, kc, jb * 512 : (jb + 1) * 512],
                            start=(kc == 0),
                            stop=(kc == NKC - 1),
                        )
                nc.vector.tensor_copy(out=kT_sb[:, pr], in_=psk[:])

            # V: [i(2048 in 16 chunks), h*dv] natural
            for ic in range(NIC):
                psv = psA.tile([128, 1024], F32, tag="pa_ps", name=f"psv{ic}")
                for kc in range(NKC):
                    lhsT = tokT_sb[:, kc, ic * 128 : (ic + 1) * 128]
                    for nb in range(2):
                        nc.tensor.matmul(
                            psv[:, nb * 512 : (nb + 1) * 512],
                            lhsT,
                            wv_sb[:, kc, nb * 512 : (nb + 1) * 512],
                            start=(kc == 0),
                            stop=(kc == NKC - 1),
                        )
                nc.vector.tensor_copy(
                    out=v_sb[:, ic, :, 0:DV],
                    in_=psv.rearrange("p (h v) -> p h v", h=H),
                )

        # Phase C inputs (DMA overlaps attention; SBUF reuses phase A space)
        pc = stack.enter_context(tc.tile_pool(name="pc", bufs=1))
        wo_sb = pc.tile([128, NKC, D], BF)
        tokres_sb = pc.tile([128, NJCH, D], F32)
        nc.sync.dma_start(wo_sb[:], wo_d[:])
        nc.sync.dma_start(tokres_sb[:], tokres_d[:])

        # ---------------- Phase B: attention per (head pair, j-block) ----------------
        # 512-wide j-blocks keep each PSUM tile to one bank, buying lookahead
        # buffers; softmax-normalize is software-pipelined one block late so the
        # PE never idles long enough to trip the HAM clock throttle.
        JB = 512
        with (
            tc.tile_pool(name="pe", bufs=8) as pe_pool,
            tc.tile_pool(name="pn", bufs=3) as pn_pool,
            tc.tile_pool(name="psS", bufs=4, space="PSUM") as psS,
            tc.tile_pool(name="psAcc", bufs=4, space="PSUM") as psAcc,
        ):

            def emit_normalize(pr, jb, acc):
                jsl = slice(jb * JB, (jb + 1) * JB)
                for hh in range(2):
                    h = 2 * pr + hh
                    hraw = pn_pool.tile(
                        [DV + 1, JB], F32, tag="hraw", name=f"hraw{h}_{jb}"
                    )
                    nc.vector.tensor_copy(out=hraw[:], in_=acc[hh][:])
                    recip = pn_pool.tile([1, JB], F32, tag="recip", name=f"recip{h}_{jb}")
                    nc.vector.reciprocal(recip[:], hraw[DV : DV + 1, :])
                    recip_bf = pn_pool.tile([1, JB], BF, tag="rbf", name=f"rbf{h}_{jb}")
                    nc.vector.tensor_copy(out=recip_bf[:], in_=recip[:])
                    ps_bc = psS.tile([DV, JB], F32, tag="sc", name=f"ps_bc{h}_{jb}")
                    nc.tensor.matmul(
                        ps_bc[:], ones_bf[:], recip_bf[:], start=True, stop=True
                    )
                    if hh == 0:
                        nc.vector.tensor_tensor(
                            multiT_sb[0:64, h // 2, jsl], hraw[0:DV, :], ps_bc[:], mult
                        )
                    else:
                        tmp64 = pn_pool.tile([DV, JB], BF, tag="tmp64", name=f"tmp{h}_{jb}")
                        nc.vector.tensor_tensor(tmp64[:], hraw[0:DV, :], ps_bc[:], mult)
                        nc.sync.dma_start(
                            out=multiT_sb[64:128, h // 2, jsl], in_=tmp64[:]
                        )

            pending = None
            for pr in range(NPAIR):
                for jb in range(2):
                    acc = [
                        psAcc.tile([DV + 1, JB], F32, tag="acc", name=f"acc{pr}_{jb}_{hh}")
                        for hh in range(2)
                    ]
                    for ic in range(NIC):
                        ps_s = [
                            psS.tile(
                                [128, JB], F32, tag="sc", name=f"ps_s{pr}_{jb}_{ic}_{hh}"
                            )
                            for hh in range(2)
                        ]
                        # scores^T for both heads; row-tiled (K=64 at partitions 0/64)
                        for hh in range(2):
                            nc.tensor.matmul(
                                ps_s[hh][:],
                                qT_sb[
                                    hh * 64 : (hh + 1) * 64, pr, ic * 128 : (ic + 1) * 128
                                ],
                                kT_sb[
                                    hh * 64 : (hh + 1) * 64, pr, jb * JB : (jb + 1) * JB
                                ],
                                start=True,
                                stop=True,
                            )
                        if ic == 2 and pending is not None:
                            emit_normalize(*pending)
                            pending = None
                        for hh in range(2):
                            eT = pe_pool.tile(
                                [128, JB], BF, tag="eT", name=f"eT{pr}_{jb}_{ic}_{hh}"
                            )
                            nc.scalar.activation(eT[:], ps_s[hh][:], Exp, scale=0.125)
                            nc.tensor.matmul(
                                acc[hh][:],
                                v_sb[:, ic, 2 * pr + hh, :],
                                eT[:],
                                start=(ic == 0),
                                stop=(ic == NIC - 1),
                            )
                    pending = (pr, jb, acc)
            emit_normalize(*pending)

        # ---------------- Phase C: out-proj + residual + LayerNorm ----------------
        with (
            tc.tile_pool(name="pC", bufs=2) as pC,
            tc.tile_pool(name="pStats", bufs=8) as pStats,
            tc.tile_pool(name="psC", bufs=2, space="PSUM") as psC,
        ):
            for jch in range(NJCH):
                ps_o = psC.tile([128, D], F32, tag="po", name=f"ps_o{jch}")
                for kc in range(NKC):
                    lhsT = multiT_sb[:, kc, jch * 128 : (jch + 1) * 128]
                    for nb in range(2):
                        nc.tensor.matmul(
                            ps_o[:, nb * 512 : (nb + 1) * 512],
                            lhsT,
                            wo_sb[:, kc, nb * 512 : (nb + 1) * 512],
                            start=(kc == 0),
                            stop=(kc == NKC - 1),
                        )
                x_sb = pC.tile([128, D], F32, tag="x", name=f"x{jch}")
                nc.vector.tensor_tensor(x_sb[:], ps_o[:], tokres_sb[:, jch, :], add)
                sum_t = pStats.tile([128, 1], F32, tag="sum", name=f"sum{jch}")
                nc.vector.reduce_sum(sum_t[:], x_sb[:], axis=AX)
                negmean = pStats.tile([128, 1], F32, tag="nm", name=f"nm{jch}")
                nc.vector.tensor_scalar_mul(negmean[:], sum_t[:], -1.0 / D)
                nc.vector.tensor_scalar_add(x_sb[:], x_sb[:], negmean[:])
                sq_sb = pC.tile([128, D], F32, tag="sq", name=f"sq{jch}")
                ssq = pStats.tile([128, 1], F32, tag="ssq", name=f"ssq{jch}")
                nc.scalar.activation(sq_sb[:], x_sb[:], Square, accum_out=ssq[:])
                std_t = pStats.tile([128, 1], F32, tag="std", name=f"std{jch}")
                nc.scalar.activation(std_t[:], ssq[:], Sqrt, bias=eps_sb[:], scale=1.0 / D)
                rstd = pStats.tile([128, 1], F32, tag="rstd", name=f"rstd{jch}")
                nc.vector.reciprocal(rstd[:], std_t[:])
                out_sb = pC.tile([128, D], F32, tag="out", name=f"out{jch}")
                nc.vector.tensor_scalar_mul(out_sb[:], x_sb[:], rstd[:])
                nc.vector.tensor_tensor(out_sb[:], out_sb[:], gamma_sb[:], mult)
                nc.vector.tensor_tensor(out_sb[:], out_sb[:], beta_sb[:], add)
                nc.sync.dma_start(out_d[:, jch], out_sb[:])

    nc.compile()
    return nc


def _prep_inputs(tokens, Wq, Wk, Wv, Wo, gamma, beta):
    """Host-side layout prep. Returns per-core input maps."""
    tokens = np.ascontiguousarray(np.asarray(tokens, dtype=np.float32))
    # weights -> [p, kc, n] with row index kc*128+p
    def rows128(a):  # [1024, N] -> [128, 8, N]
        return np.ascontiguousarray(
            a.reshape(NKC, 128, a.shape[-1]).transpose(1, 0, 2)
        )

    wq_all = rows128(np.asarray(Wq).transpose(1, 0, 2).reshape(D, H * DK).astype(BF16))
    wk_all = rows128(np.asarray(Wk).transpose(1, 0, 2).reshape(D, H * DK).astype(BF16))
    wv_all = rows128(np.asarray(Wv).transpose(1, 0, 2).reshape(D, H * DV).astype(BF16))
    wo_all = rows128(np.asarray(Wo).astype(BF16))
    gamma_bc = np.ascontiguousarray(
        np.broadcast_to(np.asarray(gamma, np.float32), (128, D))
    )
    beta_bc = np.ascontiguousarray(
        np.broadcast_to(np.asarray(beta, np.float32), (128, D))
    )

    tokT_by_b = []
    for b in range(B):
        tokT_by_b.append(rows128(tokens[b].T.astype(BF16)))  # [128, 8, 2048]

    in_maps = []
    for c in range(NCORES):
        b, jc = c // 2, c % 2
        tokT = tokT_by_b[b]
        tokTj = np.ascontiguousarray(tokT[:, :, jc * JW : (jc + 1) * JW])
        tokres = np.ascontiguousarray(
            tokens[b, jc * JW : (jc + 1) * JW]
            .reshape(NJCH, 128, D)
            .transpose(1, 0, 2)
        )
        in_maps.append(
            {
                "tokT": tokT,
                "tokTj": tokTj,
                "wq": wq_all,
                "wk": wk_all,
                "wv": wv_all,
                "wo": wo_all,
                "tokres": tokres,
                "gamma_bc": gamma_bc,
                "beta_bc": beta_bc,
            }
        )
    return in_maps


def run(inputs, trace=False, tmpdir=None):
    """Run on hardware; returns (output, BassKernelResults)."""
    from concourse.bass_utils import run_bass_kernel_spmd

    if "nc" not in _CACHE:
        _CACHE["nc"] = _build_nc()
    nc = _CACHE["nc"]
    in_maps = _prep_inputs(**inputs)
    res = run_bass_kernel_spmd(
        nc, in_maps, core_ids=list(range(NCORES)), trace=trace, tmpdir=tmpdir
    )
    out = np.empty((B, S, D), np.float32)
    for c in range(NCORES):
        b, jc = c // 2, c % 2
        o = res.results[c]["out"]  # [128, 8, 1024]
        out[b, jc * JW : (jc + 1) * JW] = (
            o.transpose(1, 0, 2).reshape(JW, D)
        )
    return out, res


def kernel(tokens, Wq, Wk, Wv, Wo, gamma, beta):
    out, _ = run(
        dict(tokens=tokens, Wq=Wq, Wk=Wk, Wv=Wv, Wo=Wo, gamma=gamma, beta=beta)
    )
    return out
